# revision 1
# baseline (speedup 1.0000x reference)
"""TRN2 Bass kernel for nn_BasicAttention (B=8, S=2048, D=1024, fp32).

out[b] = concat([x[b], softmax(x[b] @ y[b].T) @ y[b]], axis=-1)

Sharding: batch b -> NeuronCore b (8 cores, data parallel, no collectives).

Per-core pipeline (e[i,j] orientation: i on partitions, softmax on FREE axis):
  - y16 = fp16(y) via casting DMA; yT[d,j] via XBAR DMA-transpose (no PE)
  - x staged f32 in SBUF once: feeds the exact passthrough store AND the
    fp16 cast; xT[d,i] via XBAR DMA-transpose
  - MM1: e[i,j] = sum_d xT[d,i].T @ yT[d,j] -> PSUM [128,2048] (4 banks)
  - softmax: DVE free-axis max (negated) -> ACT exp(e - max) with
    per-partition bias + accum_out denominator (single pass, PSUM -> fp16)
  - aiT[j,i] via XBAR DMA-transpose (2-byte)
  - MM2: a[i,d] = sum_j aiT[j,i].T @ y16[j,d]; DVE scales by 1/den
  - PE stream is pure GEMM, software-pipelined: MM1(it), MM2(it-1), ...
"""
import sys

if '/opt/trn_rl_repo' not in sys.path:
    sys.path.insert(0, '/opt/trn_rl_repo')

import json
import numpy as np

import bass_rust
import concourse.bass as bass
import concourse.mybir as mybir
from concourse.tile import TileContext

F32 = mybir.dt.float32
F16 = mybir.dt.float16

B = 8             # batches == cores
S = 2048          # sequence length (Sx == Sy)
D = 1024          # feature dim
IT = S // 128     # 16 i-tiles
JT = S // 128     # 16 j-tiles
KT = D // 128     # 8 d-tiles (MM1 contraction)
JB = S // 512     # 4 j-chunks (MM1 psum banks)
DC = D // 512     # 2 d-chunks (MM2 psum banks)


def _legalize_waits(nc):
    """This toolchain's walrus accepts at most ONE sync-wait per
    instruction. Hoist extra waits onto single-wait NoOps inserted just
    before the offending instruction on the same engine."""
    d = json.loads(bass_rust.module_to_json_string(nc.m))
    nfix = 0
    for fn in d["functions"]:
        for bb in fn["blocks"]:
            new_insts = []
            for inst in bb["instructions"]:
                si = inst.get("sync_info")
                ow = si.get("on_wait", []) if si else []
                if len(ow) > 1:
                    for w in ow[:-1]:
                        nfix += 1
                        new_insts.append({
                            "engine": inst["engine"],
                            "ins": [], "outs": [],
                            "name": f"waitfix-{nfix}",
                            "opcode": "NoOp",
                            "sync_info": {"on_update": [], "on_wait": [w]},
                        })
                    si["on_wait"] = [ow[-1]]
                new_insts.append(inst)
            bb["instructions"] = new_insts
    nc.m = bass_rust.module_from_json_string(json.dumps(d))
    return nc


def build_attention_nc(reps=1):
    nc = bass.Bass(trn_type="TRN2", target_bir_lowering=False)
    x = nc.dram_tensor("x", [S, D], F32, kind="ExternalInput")
    y = nc.dram_tensor("y", [S, D], F32, kind="ExternalInput")
    out = nc.dram_tensor("out", [S, 2 * D], F32, kind="ExternalOutput")

    with TileContext(nc) as tc:
        with tc.tile_pool(name="persist", bufs=1) as persist, \
             tc.tile_pool(name="stage", bufs=2) as stage, \
             tc.tile_pool(name="small", bufs=3) as small, \
             tc.tile_pool(name="e_ps", bufs=1, space="PSUM") as e_pool, \
             tc.tile_pool(name="a_ps", bufs=2, space="PSUM") as a_pool:

            # ---- y prep (rep-invariant): bulk casting DMA (one per
            # j-quarter) + XBAR transposes. Per-quarter tiles keep deps
            # exact so MM1 can start as soon as quarter 0 is transposed.
            y16q = [persist.tile([128, 4, D], F16, tag=f"y16_{q}",
                                 name=f"y16_{q}")
                    for q in range(JB)]
            yTq = [persist.tile([128, KT, 512], F16, tag=f"yT_{q}",
                                name=f"yT_{q}")
                   for q in range(JB)]
            for q in range(JB):
                nc.gpsimd.dma_start(
                    out=y16q[q][:],
                    in_=y[q * 512:(q + 1) * 512, :].rearrange(
                        "(t p) d -> p t d", p=128))
                for c in range(4):
                    nc.sync.dma_start_transpose(
                        yTq[q][:, :, c * 128:(c + 1) * 128],
                        y16q[q][:, c, :])

            # ---- x prep helper: one HBM read feeds passthrough + cast ----
            xTs = [persist.tile([128, KT, 128], F16, tag=f"xT_{it}",
                                name=f"xT_{it}")
                   for it in range(IT)]

            def load_x(it):
                r0 = it * 128
                x32 = stage.tile([128, D], F32, tag="x32")
                nc.gpsimd.dma_start(out=x32[:], in_=x[r0:r0 + 128, :])
                nc.gpsimd.dma_start(out=out[r0:r0 + 128, 0:D], in_=x32[:])
                return x32

            def cast_x(it, x32):
                x16 = stage.tile([128, D], F16, tag="x16")
                nc.vector.tensor_copy(out=x16[:], in_=x32[:])
                nc.sync.dma_start_transpose(xTs[it][:], x16[:])

            x32p = load_x(0)
            cast_x(0, x32p)
            prevs = []
            for _rep in range(reps):

                def do_mm2(prev):
                    it, aiT, rden = prev
                    a = a_pool.tile([128, D], F32, tag="a")
                    for dc in range(DC):
                        dsl = slice(dc * 512, (dc + 1) * 512)
                        for jt in range(JT):
                            nc.tensor.matmul(
                                a[:, dsl], aiT[:, jt, :],
                                y16q[jt // 4][:, jt % 4, dsl],
                                start=(jt == 0), stop=(jt == JT - 1))
                    aout = stage.tile([128, D], F32, tag="aout")
                    for dc in range(DC):
                        dsl = slice(dc * 512, (dc + 1) * 512)
                        nc.vector.tensor_scalar_mul(aout[:, dsl], a[:, dsl], rden[:])
                    r0 = it * 128
                    nc.gpsimd.dma_start(out=out[r0:r0 + 128, D:2 * D], in_=aout[:])

                for it in range(IT):
                    # ---- MM1: e[i-tile, all j] into 4 PSUM banks ----
                    e = e_pool.tile([128, S], F32, tag="e")
                    for jb in range(JB):
                        jsl = slice(jb * 512, (jb + 1) * 512)
                        for kt in range(KT):
                            nc.tensor.matmul(
                                e[:, jsl],
                                xTs[it][:, kt, :],
                                yTq[jb][:, kt, :],
                                start=(kt == 0), stop=(kt == KT - 1))
                    # prefetch next x rows early (slow DMA part only); at the
                    # last i-tile, prefetch the NEXT rep's first x so the PE
                    # stream flows across the rep boundary without draining
                    nit = it + 1 if it + 1 < IT else (0 if _rep + 1 < reps else None)
                    if nit is not None:
                        x32n = load_x(nit)
                    # ---- softmax along free axis (issued before MM2 so the
                    # DVE/ACT queues aren't blocked behind later-dependent ops)
                    negmax = small.tile([128, 1], F32, tag="negmax")
                    nc.vector.tensor_reduce(
                        out=negmax[:], in_=e[:],
                        axis=mybir.AxisListType.X, op=mybir.AluOpType.max,
                        negate=True)
                    ai = stage.tile([128, S], F16, tag="ai")
                    den = small.tile([128, 1], F32, tag="den")
                    nc.scalar.activation(
                        out=ai[:], in_=e[:],
                        func=mybir.ActivationFunctionType.Exp,
                        bias=negmax[:], accum_out=den[:])
                    rden = small.tile([128, 1], F32, tag="rden")
                    nc.vector.reciprocal(rden[:], den[:])
                    # xT(it+1) first on the XBAR queue (needed at MM1(it+1),
                    # one slot before aiT(it) is needed at MM2(it))
                    if nit is not None:
                        cast_x(nit, x32n)
                    aiT = stage.tile([128, JT, 128], F16, tag="aiT", bufs=3)
                    nc.sync.dma_start_transpose(aiT[:], ai[:])
                    prevs.append((it, aiT, rden))
                    # ---- MM2 two i-tiles behind: the softmax->aiT chain gets
                    # two full PE slots of slack before MM2 consumes it
                    if len(prevs) > 2:
                        do_mm2(prevs.pop(0))
            for p in prevs:
                do_mm2(p)
    return nc


class _Runner:
    """Compile once; run with device-resident sharded inputs via PJRT."""

    def __init__(self, reps=1):
        import jax
        from jax.sharding import Mesh, PartitionSpec, NamedSharding
        from jax.experimental.shard_map import shard_map
        from concourse import bass2jax
        from concourse.bass2jax import _bass_exec_p, install_neuronx_cc_hook

        install_neuronx_cc_hook()
        nc = _legalize_waits(build_attention_nc(reps=reps))
        self.nc = nc
        partition_name = nc.partition_id_tensor.name if nc.partition_id_tensor else None
        in_names, out_names, out_avals = [], [], []
        zero_specs = []
        for alloc in nc.m.functions[0].allocations:
            if not isinstance(alloc, mybir.MemoryLocationSet):
                continue
            name = alloc.memorylocations[0].name
            if alloc.kind == "ExternalInput":
                if name != partition_name:
                    in_names.append(name)
            elif alloc.kind == "ExternalOutput":
                out_names.append(name)
                shape = tuple(alloc.tensor_shape)
                dtype = mybir.dt.np(alloc.dtype)
                out_avals.append(jax.core.ShapedArray(shape, dtype))
                zero_specs.append((shape, dtype))
        self.in_names, self.out_names, self.out_avals = in_names, out_names, out_avals
        n_params, n_outs = len(in_names), len(out_names)

        def _body(*args):
            operands = list(args)
            if partition_name is not None:
                operands.append(bass2jax.partition_id_tensor())
            outs = _bass_exec_p.bind(
                *operands,
                out_avals=tuple(out_avals),
                in_names=tuple(in_names + out_names
                               + ([partition_name] if partition_name else [])),
                out_names=tuple(out_names),
                lowering_input_output_aliases=(),
                sim_require_finite=True,
                sim_require_nnan=True,
                nc=nc,
            )
            return tuple(outs)

        devices = jax.devices()[:B]
        self.mesh = Mesh(np.asarray(devices), ("core",))
        in_specs = (PartitionSpec("core"),) * (n_params + n_outs)
        out_specs = (PartitionSpec("core"),) * n_outs
        donate = tuple(range(n_params, n_params + n_outs))
        self.sharded = jax.jit(
            shard_map(_body, mesh=self.mesh, in_specs=in_specs,
                      out_specs=out_specs, check_rep=False),
            donate_argnums=donate, keep_unused=True)
        self.sharding = NamedSharding(self.mesh, PartitionSpec("core"))
        import jax.numpy as jnp
        zshapes = [(B * s[0], *s[1:]) for s, _ in zero_specs]
        zdtypes = [dt for _, dt in zero_specs]
        self._mk_zeros = jax.jit(
            lambda: tuple(jnp.zeros(s, d) for s, d in zip(zshapes, zdtypes)),
            out_shardings=tuple(self.sharding for _ in zshapes))
        self._jax = jax

    def put_inputs(self, per_core_maps):
        concat = [np.concatenate([np.asarray(m[name]) for m in per_core_maps], axis=0)
                  for name in self.in_names]
        return [self._jax.device_put(a, self.sharding) for a in concat]

    def run_raw(self, in_dev):
        outs = self.sharded(*in_dev, *self._mk_zeros())
        self._jax.block_until_ready(outs)
        return outs

    def run(self, per_core_maps):
        outs = self.run_raw(self.put_inputs(per_core_maps))
        res = []
        for c in range(B):
            res.append({
                name: np.asarray(outs[i]).reshape(B, *self.out_avals[i].shape)[c]
                for i, name in enumerate(self.out_names)})
        return res


_RUNNER_CACHE = {}


def _get_runner(reps=1):
    if reps not in _RUNNER_CACHE:
        _RUNNER_CACHE[reps] = _Runner(reps=reps)
    return _RUNNER_CACHE[reps]


def kernel(x: np.ndarray, y: np.ndarray) -> np.ndarray:
    """Full-input entry point: x [8,2048,1024] f32, y [8,2048,1024] f32
    -> out [8,2048,2048] f32."""
    x = np.asarray(x, dtype=np.float32)
    y = np.asarray(y, dtype=np.float32)
    assert x.shape == (B, S, D) and y.shape == (B, S, D)
    r = _get_runner(reps=1)
    maps = [{"x": x[c], "y": y[c]} for c in range(B)]
    res = r.run(maps)
    return np.stack([res[c]["out"] for c in range(B)])



# revision 2
# speedup vs baseline: 1.0390x; 1.0390x over previous
"""TRN2 Bass kernel for nn_BasicAttention (B=8, S=2048, D=1024, fp32).

out[b] = concat([x[b], softmax(x[b] @ y[b].T) @ y[b]], axis=-1)

Sharding: batch b -> NeuronCore b (8 cores, data parallel, no collectives).

fp8 DoubleRow design (PE runs e4m3 DR matmuls at ~4x fp16 row rate):
  - hi/lo split: t16 = f16(t); th8 = e4m3(t16) [exact DMA cast];
    th16 = f16(th8) [exact]; tl8 = e4m3(t16 - th16). Then
    t ~= th8 + tl8 with ~fp16-level product accuracy.
  - MM1: e = xh@yh' + xh@yl' + xl@yh'  (3 DR terms, drops lo@lo)
  - softmax on free axis: per-bank partial maxes (overlap with MM1 tail),
    ACT exp with bias + accum_out denominator -> ai16
  - aiT via XBAR (2-byte), cast to e4m3 on Pool
  - MM2: a = ai8@yh + ai8@yl (2 DR terms); ACT scales by 1/den
  - kt-pair-outer ordering amortizes DR weight loads (4-8 consecutive
    matmuls reuse each stationary operand)
"""
import sys

if '/opt/trn_rl_repo' not in sys.path:
    sys.path.insert(0, '/opt/trn_rl_repo')

import json
import numpy as np

import bass_rust
import concourse.bass as bass
import concourse.mybir as mybir
from concourse.tile import TileContext

F32 = mybir.dt.float32
F16 = mybir.dt.float16
F8 = mybir.dt.float8e4
DR = mybir.MatmulPerfMode.DoubleRow

B = 8             # batches == cores
S = 2048          # sequence length (Sx == Sy)
D = 1024          # feature dim
IT = S // 128     # 16 i-tiles
JT = S // 128     # 16 j-tiles
KT = D // 128     # 8 d-tiles
KP = KT // 2      # 4 d-pair tiles (DR contraction pairs)
JB = S // 512     # 4 j-chunks (MM1 psum banks)
JTP = JT // 2     # 8 j-pair tiles (MM2 DR contraction pairs)
DC = D // 512     # 2 d-chunks (MM2 psum banks)


def _legalize_waits(nc):
    """This toolchain's walrus accepts at most ONE sync-wait per
    instruction. Hoist extra waits onto single-wait NoOps inserted just
    before the offending instruction on the same engine."""
    d = json.loads(bass_rust.module_to_json_string(nc.m))
    nfix = 0
    for fn in d["functions"]:
        for bb in fn["blocks"]:
            new_insts = []
            for inst in bb["instructions"]:
                si = inst.get("sync_info")
                ow = si.get("on_wait", []) if si else []
                if len(ow) > 1:
                    for w in ow[:-1]:
                        nfix += 1
                        new_insts.append({
                            "engine": inst["engine"],
                            "ins": [], "outs": [],
                            "name": f"waitfix-{nfix}",
                            "opcode": "NoOp",
                            "sync_info": {"on_update": [], "on_wait": [w]},
                        })
                    si["on_wait"] = [ow[-1]]
                new_insts.append(inst)
            bb["instructions"] = new_insts
    nc.m = bass_rust.module_from_json_string(json.dumps(d))
    return nc


def build_attention_nc(reps=1):
    nc = bass.Bass(trn_type="TRN2", target_bir_lowering=False)
    x = nc.dram_tensor("x", [S, D], F32, kind="ExternalInput")
    y = nc.dram_tensor("y", [S, D], F32, kind="ExternalInput")
    out = nc.dram_tensor("out", [S, 2 * D], F32, kind="ExternalOutput")

    with TileContext(nc) as tc:
        with tc.tile_pool(name="persist", bufs=1) as persist, \
             tc.tile_pool(name="ystage", bufs=2) as ystage, \
             tc.tile_pool(name="stage", bufs=2) as stage, \
             tc.tile_pool(name="small", bufs=3) as small, \
             tc.tile_pool(name="e_ps", bufs=1, space="PSUM") as e_pool, \
             tc.tile_pool(name="a_ps", bufs=2, space="PSUM") as a_pool:

            # ---- y prep (rep-invariant) ----
            # yT8h/yT8l [128, KT, 512] per quarter: MM1 moving operands.
            # y8h/y8l [128, 2, D] per j-pair: MM2 moving operands.
            yT8h = [persist.tile([128, KT, 512], F8, tag=f"yT8h_{q}",
                                 name=f"yT8h_{q}") for q in range(JB)]
            yT8l = [persist.tile([128, KT, 512], F8, tag=f"yT8l_{q}",
                                 name=f"yT8l_{q}") for q in range(JB)]
            y8h = [persist.tile([128, 2, D], F8, tag=f"y8h_{p}",
                                name=f"y8h_{p}") for p in range(JTP)]
            y8l = [persist.tile([128, 2, D], F8, tag=f"y8l_{p}",
                                name=f"y8l_{p}") for p in range(JTP)]

            for q in range(JB):
                y16 = ystage.tile([128, 4, D], F16, tag="y16")
                nc.gpsimd.dma_start(
                    out=y16[:],
                    in_=y[q * 512:(q + 1) * 512, :].rearrange(
                        "(t p) d -> p t d", p=128))
                # transposed side
                yT16 = ystage.tile([128, KT, 512], F16, tag="yT16")
                for c in range(4):
                    nc.sync.dma_start_transpose(
                        yT16[:, :, c * 128:(c + 1) * 128], y16[:, c, :])
                nc.gpsimd.dma_start(out=yT8h[q][:], in_=yT16[:])
                yTh16 = ystage.tile([128, KT, 512], F16, tag="yTh16")
                nc.gpsimd.dma_start(out=yTh16[:], in_=yT8h[q][:])
                nc.gpsimd.tensor_tensor(yT8l[q][:], yT16[:], yTh16[:],
                                        mybir.AluOpType.subtract)
                # untransposed side (already in DR pair layout)
                for m in range(2):
                    p = 2 * q + m
                    nc.gpsimd.dma_start(out=y8h[p][:],
                                        in_=y16[:, 2 * m:2 * m + 2, :])
                    yh16 = ystage.tile([128, 2, D], F16, tag="yh16")
                    nc.gpsimd.dma_start(out=yh16[:], in_=y8h[p][:])
                    nc.gpsimd.tensor_tensor(y8l[p][:],
                                            y16[:, 2 * m:2 * m + 2, :],
                                            yh16[:], mybir.AluOpType.subtract)

            # ---- x prep helpers ----
            xT8h = [persist.tile([128, KT, 128], F8, tag=f"xT8h_{it}",
                                 name=f"xT8h_{it}") for it in range(IT)]
            xT8l = [persist.tile([128, KT, 128], F8, tag=f"xT8l_{it}",
                                 name=f"xT8l_{it}") for it in range(IT)]

            def load_x(it):
                r0 = it * 128
                x32 = stage.tile([128, D], F32, tag="x32")
                nc.gpsimd.dma_start(out=x32[:], in_=x[r0:r0 + 128, :])
                nc.gpsimd.dma_start(out=out[r0:r0 + 128, 0:D], in_=x32[:])
                return x32

            def cast_x(it, x32):
                x16 = stage.tile([128, D], F16, tag="x16")
                nc.vector.tensor_copy(out=x16[:], in_=x32[:])
                xT16 = stage.tile([128, KT, 128], F16, tag="xT16")
                nc.sync.dma_start_transpose(xT16[:], x16[:])
                nc.gpsimd.dma_start(out=xT8h[it][:], in_=xT16[:])
                xTh16 = stage.tile([128, KT, 128], F16, tag="xTh16")
                nc.gpsimd.dma_start(out=xTh16[:], in_=xT8h[it][:])
                nc.vector.tensor_tensor(xT8l[it][:], xT16[:], xTh16[:],
                                        mybir.AluOpType.subtract)

            x32p = load_x(0)
            cast_x(0, x32p)
            prevs = []
            for _rep in range(reps):

                def do_mm2(prev):
                    it, aiT8, rden = prev
                    a = a_pool.tile([128, D], F32, tag="a")
                    for jtp in range(JTP):
                        lhsT = aiT8[:, 2 * jtp:2 * jtp + 2, :]
                        for term, ymat in ((0, y8h), (1, y8l)):
                            for dc in range(DC):
                                dsl = slice(dc * 512, (dc + 1) * 512)
                                nc.tensor.matmul(
                                    a[:, dsl], lhsT, ymat[jtp][:, :, dsl],
                                    start=(jtp == 0 and term == 0),
                                    stop=(jtp == JTP - 1 and term == 1),
                                    perf_mode=DR)
                    aout = stage.tile([128, D], F32, tag="aout")
                    nc.scalar.mul(aout[:], a[:], rden[:])
                    r0 = it * 128
                    nc.gpsimd.dma_start(out=out[r0:r0 + 128, D:2 * D],
                                        in_=aout[:])

                for it in range(IT):
                    # ---- MM1: e[i-tile, all j] into 4 PSUM banks, 3 DR
                    # terms, kp-outer for stationary reuse ----
                    e = e_pool.tile([128, S], F32, tag="e")
                    for kp in range(KP):
                        ksl = slice(2 * kp, 2 * kp + 2)
                        xh = xT8h[it][:, ksl, :]
                        xl = xT8l[it][:, ksl, :]
                        for jb in range(JB):
                            jsl = slice(jb * 512, (jb + 1) * 512)
                            nc.tensor.matmul(
                                e[:, jsl], xh, yT8h[jb][:, ksl, :],
                                start=(kp == 0), stop=False, perf_mode=DR)
                        for jb in range(JB):
                            jsl = slice(jb * 512, (jb + 1) * 512)
                            nc.tensor.matmul(
                                e[:, jsl], xh, yT8l[jb][:, ksl, :],
                                start=False, stop=False, perf_mode=DR)
                        for jb in range(JB):
                            jsl = slice(jb * 512, (jb + 1) * 512)
                            nc.tensor.matmul(
                                e[:, jsl], xl, yT8h[jb][:, ksl, :],
                                start=False, stop=(kp == KP - 1),
                                perf_mode=DR)
                    # prefetch next x rows early; at the last i-tile,
                    # prefetch the NEXT rep's first x
                    nit = it + 1 if it + 1 < IT else (0 if _rep + 1 < reps else None)
                    if nit is not None:
                        x32n = load_x(nit)
                    # ---- softmax along free axis; per-bank partial maxes
                    # so exp can start right after MM1's last bank closes
                    pmax = small.tile([128, JB], F32, tag="pmax")
                    for jb in range(JB):
                        jsl = slice(jb * 512, (jb + 1) * 512)
                        nc.vector.tensor_reduce(
                            out=pmax[:, jb:jb + 1], in_=e[:, jsl],
                            axis=mybir.AxisListType.X, op=mybir.AluOpType.max)
                    negmax = small.tile([128, 1], F32, tag="negmax")
                    nc.vector.tensor_reduce(
                        out=negmax[:], in_=pmax[:],
                        axis=mybir.AxisListType.X, op=mybir.AluOpType.max,
                        negate=True)
                    ai = stage.tile([128, S], F16, tag="ai")
                    den = small.tile([128, 1], F32, tag="den")
                    nc.scalar.activation(
                        out=ai[:], in_=e[:],
                        func=mybir.ActivationFunctionType.Exp,
                        bias=negmax[:], accum_out=den[:])
                    rden = small.tile([128, 1], F32, tag="rden")
                    nc.vector.reciprocal(rden[:], den[:])
                    # xT(it+1) on the XBAR/cast queues before aiT(it)
                    if nit is not None:
                        cast_x(nit, x32n)
                    aiT16 = stage.tile([128, JT, 128], F16, tag="aiT16")
                    nc.sync.dma_start_transpose(aiT16[:], ai[:])
                    aiT8 = stage.tile([128, JT, 128], F8, tag="aiT8", bufs=3)
                    nc.gpsimd.tensor_copy(out=aiT8[:], in_=aiT16[:])
                    prevs.append((it, aiT8, rden))
                    # ---- MM2 two i-tiles behind ----
                    if len(prevs) > 2:
                        do_mm2(prevs.pop(0))
            for p in prevs:
                do_mm2(p)
    return nc


class _Runner:
    """Compile once; run with device-resident sharded inputs via PJRT."""

    def __init__(self, reps=1):
        import jax
        from jax.sharding import Mesh, PartitionSpec, NamedSharding
        from jax.experimental.shard_map import shard_map
        from concourse import bass2jax
        from concourse.bass2jax import _bass_exec_p, install_neuronx_cc_hook

        install_neuronx_cc_hook()
        nc = _legalize_waits(build_attention_nc(reps=reps))
        self.nc = nc
        partition_name = nc.partition_id_tensor.name if nc.partition_id_tensor else None
        in_names, out_names, out_avals = [], [], []
        zero_specs = []
        for alloc in nc.m.functions[0].allocations:
            if not isinstance(alloc, mybir.MemoryLocationSet):
                continue
            name = alloc.memorylocations[0].name
            if alloc.kind == "ExternalInput":
                if name != partition_name:
                    in_names.append(name)
            elif alloc.kind == "ExternalOutput":
                out_names.append(name)
                shape = tuple(alloc.tensor_shape)
                dtype = mybir.dt.np(alloc.dtype)
                out_avals.append(jax.core.ShapedArray(shape, dtype))
                zero_specs.append((shape, dtype))
        self.in_names, self.out_names, self.out_avals = in_names, out_names, out_avals
        n_params, n_outs = len(in_names), len(out_names)

        def _body(*args):
            operands = list(args)
            if partition_name is not None:
                operands.append(bass2jax.partition_id_tensor())
            outs = _bass_exec_p.bind(
                *operands,
                out_avals=tuple(out_avals),
                in_names=tuple(in_names + out_names
                               + ([partition_name] if partition_name else [])),
                out_names=tuple(out_names),
                lowering_input_output_aliases=(),
                sim_require_finite=True,
                sim_require_nnan=True,
                nc=nc,
            )
            return tuple(outs)

        devices = jax.devices()[:B]
        self.mesh = Mesh(np.asarray(devices), ("core",))
        in_specs = (PartitionSpec("core"),) * (n_params + n_outs)
        out_specs = (PartitionSpec("core"),) * n_outs
        donate = tuple(range(n_params, n_params + n_outs))
        self.sharded = jax.jit(
            shard_map(_body, mesh=self.mesh, in_specs=in_specs,
                      out_specs=out_specs, check_rep=False),
            donate_argnums=donate, keep_unused=True)
        self.sharding = NamedSharding(self.mesh, PartitionSpec("core"))
        import jax.numpy as jnp
        zshapes = [(B * s[0], *s[1:]) for s, _ in zero_specs]
        zdtypes = [dt for _, dt in zero_specs]
        self._mk_zeros = jax.jit(
            lambda: tuple(jnp.zeros(s, d) for s, d in zip(zshapes, zdtypes)),
            out_shardings=tuple(self.sharding for _ in zshapes))
        self._jax = jax

    def put_inputs(self, per_core_maps):
        concat = [np.concatenate([np.asarray(m[name]) for m in per_core_maps], axis=0)
                  for name in self.in_names]
        return [self._jax.device_put(a, self.sharding) for a in concat]

    def run_raw(self, in_dev):
        outs = self.sharded(*in_dev, *self._mk_zeros())
        self._jax.block_until_ready(outs)
        return outs

    def run(self, per_core_maps):
        outs = self.run_raw(self.put_inputs(per_core_maps))
        res = []
        for c in range(B):
            res.append({
                name: np.asarray(outs[i]).reshape(B, *self.out_avals[i].shape)[c]
                for i, name in enumerate(self.out_names)})
        return res


_RUNNER_CACHE = {}


def _get_runner(reps=1):
    if reps not in _RUNNER_CACHE:
        _RUNNER_CACHE[reps] = _Runner(reps=reps)
    return _RUNNER_CACHE[reps]


def kernel(x: np.ndarray, y: np.ndarray) -> np.ndarray:
    """Full-input entry point: x [8,2048,1024] f32, y [8,2048,1024] f32
    -> out [8,2048,2048] f32."""
    x = np.asarray(x, dtype=np.float32)
    y = np.asarray(y, dtype=np.float32)
    assert x.shape == (B, S, D) and y.shape == (B, S, D)
    r = _get_runner(reps=1)
    maps = [{"x": x[c], "y": y[c]} for c in range(B)]
    res = r.run(maps)
    return np.stack([res[c]["out"] for c in range(B)])
